# revision 12
# baseline (speedup 1.0000x reference)
import math

import numpy as np
import jax
import jax.numpy as jnp

# Problem: nn_CGABlock_38087769981516 — data-parallel over 8 NeuronCores.
B, C_IN, C_OUT, V = 512, 64, 64, 25
MID = C_IN // 8
INTER = C_OUT // 2
BN_EPS = 1e-5
N_CORES = 8
BS = B // N_CORES

_PREC = jax.lax.Precision.HIGHEST
_RSQV = 1.0 / math.sqrt(V)

# The fully-fused graph trips a PGTiling internal compiler error
# (NCC_IPCC901); the block is split into a "front" graph and small back
# stages that each compile cleanly. Intermediates stay device-resident.


def _front(x, w1, b1, w2, b2, w3, b3, dw, db, edge_w, edge_b, att_w, att_b):
    x1 = jnp.matmul(w1[None], x, precision=_PREC) + b1[:, None]
    x2 = jnp.matmul(w2[None], x, precision=_PREC) + b2[:, None]
    x3 = jnp.matmul(w3[None], x, precision=_PREC) + b3[:, None]
    # Grouped pairwise-diff conv, exact rank-1 form. Pairing: group g reads
    # channels (2g, 2g+1) of concat([d1, d2]) — g<4 from d1, g>=4 from d2.
    x1r = x1.reshape(-1, MID // 2, 2, V)
    x2r = x2.reshape(-1, MID // 2, 2, V)
    dwa = dw[:MID // 2].reshape(1, MID // 2, 2, 1)
    dwb = dw[MID // 2:].reshape(1, MID // 2, 2, 1)
    f1 = jnp.concatenate([(x1r * dwa).sum(2), (x2r * dwb).sum(2)], axis=1)
    f2 = jnp.concatenate([(x2r * dwa).sum(2), (x1r * dwb).sum(2)], axis=1)
    A_dyn = jnp.tanh(f1[:, :, :, None] - f2[:, :, None, :]
                     + db[None, :, None, None]).reshape(-1, MID, V * V)
    A_mix = jnp.matmul(edge_w[None], A_dyn, precision=_PREC) \
        + edge_b[None, :, None]
    att = jnp.tanh((x1[:, :, :, None] * x2[:, :, None, :]) * _RSQV) \
        .reshape(-1, MID, V * V)
    att_m = jnp.matmul(att_w[None], att, precision=_PREC) \
        + att_b[None, :, None]
    return x3, A_mix, att_m.reshape(-1, C_OUT, V, V)


def _s4_xatt(x3, att_m):
    bs = x3.shape[0]
    return jnp.matmul(x3.reshape(bs * C_OUT, 1, V),
                      att_m.reshape(bs * C_OUT, V, V),
                      precision=_PREC).reshape(bs, C_OUT, V)


def _s5_xgcn(A_mix, x3, A_static, alpha):
    A_out = A_static.reshape(1, 1, V * V) + alpha * A_mix
    bs = x3.shape[0]
    return jnp.matmul(A_out.reshape(bs * C_OUT, V, V),
                      x3.reshape(bs * C_OUT, V, 1),
                      precision=_PREC).reshape(bs, C_OUT, V)


def _s6_final(x, x_att, x_gcn0, cc1_w, cc1_b, bn_g, bn_b, bn_m, bn_v,
              cc2_w, cc2_b, cs_w, cs_b):
    xm = x_att.mean(-1, keepdims=True)
    h = jnp.matmul(cc1_w[None], xm, precision=_PREC) + cc1_b[:, None]
    h = (h - bn_m[:, None]) * (bn_g / jnp.sqrt(bn_v + BN_EPS))[:, None] \
        + bn_b[:, None]
    h = jax.nn.gelu(h, approximate=False)
    c_att = jax.nn.sigmoid(
        jnp.matmul(cc2_w[None], h, precision=_PREC) + cc2_b[:, None])
    x_gcn = x_gcn0 * c_att
    s_att = jax.nn.sigmoid(
        jnp.matmul(cs_w[None], x_gcn, precision=_PREC) + cs_b[:, None])
    return x_gcn + x_att * s_att + x


_stages = None


def _get_stages():
    global _stages
    if _stages is None:
        devs = jax.devices()[:N_CORES]
        pm = lambda f, nrep: jax.pmap(
            f, in_axes=tuple([0] * (f.__code__.co_argcount - nrep)
                             + [None] * nrep), devices=devs)
        _stages = {
            'front': pm(_front, 12),
            's4': pm(_s4_xatt, 0),
            's5': pm(_s5_xgcn, 2),
            's6': pm(_s6_final, 10),
        }
    return _stages


def kernel(**inputs):
    st = _get_stages()
    g = {k: np.asarray(v, dtype=np.float32) for k, v in inputs.items()}
    xs = g['x'].reshape(N_CORES, BS, C_IN, V)

    x3, A_mix, att_m = st['front'](
        xs, g['w1'], g['b1'], g['w2'], g['b2'], g['w3'], g['b3'],
        g['diff_w'], g['diff_b'], g['edge_w'], g['edge_b'],
        g['att_w'], g['att_b'])
    x_att = st['s4'](x3, att_m)
    x_gcn0 = st['s5'](A_mix, x3, g['A_static'], g['alpha'])
    out = st['s6'](xs, x_att, x_gcn0, g['cc1_w'], g['cc1_b'], g['bn_g'],
                   g['bn_b'], g['bn_m'], g['bn_v'], g['cc2_w'], g['cc2_b'],
                   g['cs_w'], g['cs_b'])
    return np.asarray(out).reshape(B, C_OUT, V).astype(np.float32)


# revision 13
# speedup vs baseline: 1.0102x; 1.0102x over previous
import math

import numpy as np
import jax
import jax.numpy as jnp

# Problem: nn_CGABlock_38087769981516 — data-parallel over 8 NeuronCores.
B, C_IN, C_OUT, V = 512, 64, 64, 25
MID = C_IN // 8
INTER = C_OUT // 2
BN_EPS = 1e-5
N_CORES = 8
BS = B // N_CORES

_PREC = jax.lax.Precision.HIGHEST
_RSQV = 1.0 / math.sqrt(V)

# The fully-fused graph trips a PGTiling internal compiler error
# (NCC_IPCC901); the block is split into a "front" graph and small back
# stages that each compile cleanly. Intermediates stay device-resident.


def _front(x, w1, b1, w2, b2, w3, b3, dw, db, edge_w, edge_b, att_w, att_b, A_static, alpha):
    x1 = jnp.matmul(w1[None], x, precision=_PREC) + b1[:, None]
    x2 = jnp.matmul(w2[None], x, precision=_PREC) + b2[:, None]
    x3 = jnp.matmul(w3[None], x, precision=_PREC) + b3[:, None]
    # Grouped pairwise-diff conv, exact rank-1 form. Pairing: group g reads
    # channels (2g, 2g+1) of concat([d1, d2]) — g<4 from d1, g>=4 from d2.
    x1r = x1.reshape(-1, MID // 2, 2, V)
    x2r = x2.reshape(-1, MID // 2, 2, V)
    dwa = dw[:MID // 2].reshape(1, MID // 2, 2, 1)
    dwb = dw[MID // 2:].reshape(1, MID // 2, 2, 1)
    f1 = jnp.concatenate([(x1r * dwa).sum(2), (x2r * dwb).sum(2)], axis=1)
    f2 = jnp.concatenate([(x2r * dwa).sum(2), (x1r * dwb).sum(2)], axis=1)
    A_dyn = jnp.tanh(f1[:, :, :, None] - f2[:, :, None, :]
                     + db[None, :, None, None]).reshape(-1, MID, V * V)
    A_mix = jnp.matmul(edge_w[None], A_dyn, precision=_PREC) \
        + edge_b[None, :, None]
    att = jnp.tanh((x1[:, :, :, None] * x2[:, :, None, :]) * _RSQV) \
        .reshape(-1, MID, V * V)
    att_m = jnp.matmul(att_w[None], att, precision=_PREC) \
        + att_b[None, :, None]
    bs = x3.shape[0]
    x_att = jnp.matmul(x3.reshape(bs * C_OUT, 1, V),
                       att_m.reshape(bs * C_OUT, V, V),
                       precision=_PREC).reshape(bs, C_OUT, V)
    A_out = A_static.reshape(1, 1, V * V) + alpha * A_mix
    x_gcn0 = jnp.matmul(A_out.reshape(bs * C_OUT, V, V),
                        x3.reshape(bs * C_OUT, V, 1),
                        precision=_PREC).reshape(bs, C_OUT, V)
    return x_att, x_gcn0


def _s4_xatt(x3, att_m):
    bs = x3.shape[0]
    return jnp.matmul(x3.reshape(bs * C_OUT, 1, V),
                      att_m.reshape(bs * C_OUT, V, V),
                      precision=_PREC).reshape(bs, C_OUT, V)


def _s5_xgcn(A_mix, x3, A_static, alpha):
    A_out = A_static.reshape(1, 1, V * V) + alpha * A_mix
    bs = x3.shape[0]
    return jnp.matmul(A_out.reshape(bs * C_OUT, V, V),
                      x3.reshape(bs * C_OUT, V, 1),
                      precision=_PREC).reshape(bs, C_OUT, V)


def _s6_final(x, x_att, x_gcn0, cc1_w, cc1_b, bn_g, bn_b, bn_m, bn_v,
              cc2_w, cc2_b, cs_w, cs_b):
    xm = x_att.mean(-1, keepdims=True)
    h = jnp.matmul(cc1_w[None], xm, precision=_PREC) + cc1_b[:, None]
    h = (h - bn_m[:, None]) * (bn_g / jnp.sqrt(bn_v + BN_EPS))[:, None] \
        + bn_b[:, None]
    h = jax.nn.gelu(h, approximate=False)
    c_att = jax.nn.sigmoid(
        jnp.matmul(cc2_w[None], h, precision=_PREC) + cc2_b[:, None])
    x_gcn = x_gcn0 * c_att
    s_att = jax.nn.sigmoid(
        jnp.matmul(cs_w[None], x_gcn, precision=_PREC) + cs_b[:, None])
    return x_gcn + x_att * s_att + x


_stages = None


def _get_stages():
    global _stages
    if _stages is None:
        devs = jax.devices()[:N_CORES]
        pm = lambda f, nrep: jax.pmap(
            f, in_axes=tuple([0] * (f.__code__.co_argcount - nrep)
                             + [None] * nrep), devices=devs)
        _stages = {
            'front': pm(_front, 14),
            's6': pm(_s6_final, 10),
        }
    return _stages


def kernel(**inputs):
    st = _get_stages()
    g = {k: np.asarray(v, dtype=np.float32) for k, v in inputs.items()}
    xs = g['x'].reshape(N_CORES, BS, C_IN, V)

    x_att, x_gcn0 = st['front'](
        xs, g['w1'], g['b1'], g['w2'], g['b2'], g['w3'], g['b3'],
        g['diff_w'], g['diff_b'], g['edge_w'], g['edge_b'],
        g['att_w'], g['att_b'], g['A_static'], g['alpha'])
    out = st['s6'](xs, x_att, x_gcn0, g['cc1_w'], g['cc1_b'], g['bn_g'],
                   g['bn_b'], g['bn_m'], g['bn_v'], g['cc2_w'], g['cc2_b'],
                   g['cs_w'], g['cs_b'])
    return np.asarray(out).reshape(B, C_OUT, V).astype(np.float32)
